# revision 2
# baseline (speedup 1.0000x reference)
"""BitLinear v2: PE does ONLY the 1024 main matmuls; activation transposes
run on the DMA xbar (one batched InstDmaTransposeAnt per m-tile, bit-exact),
PSUM is fully double-buffered (4 tags x 2 bufs = 8 banks), bias-add moves to
the Pool engine, and the transpose pipeline runs 2 tiles ahead of the MMs.

Math is bit-faithful to the jax reference (same op/engine choices as the
v1 kernel for every rounding-sensitive step)."""

from contextlib import ExitStack

import numpy as np
import ml_dtypes

import concourse.bass as bass
import concourse.mybir as mybir
import concourse.tile as tile
from concourse import bacc
from concourse.bass_utils import run_bass_kernel_spmd

P = 128
D = 2048               # in_features
O = 2048               # out_features
N_CORES = 8
B, S = 4, 4096
M_TOTAL = B * S
M_CORE = M_TOTAL // N_CORES   # 2048
NK = D // P            # 16 contraction blocks
O_CHUNK = 512
N_OCH = O // O_CHUNK   # 4
MAGIC = 12582912.0     # 1.5 * 2^23
QP = 127.0
AHEAD = 2              # transpose pipeline depth (tiles)


def build_nc(m_core=M_CORE, repeats=1, variant="main"):
    m_tiles = m_core // P
    nc = bacc.Bacc(None)
    x = nc.declare_dram_parameter("x", [m_core, D], mybir.dt.float32, isOutput=False)
    wT = nc.declare_dram_parameter("wT", [D, O], mybir.dt.bfloat16, isOutput=False)
    bias = nc.declare_dram_parameter("bias", [O], mybir.dt.float32, isOutput=False)
    ws = nc.declare_dram_parameter("ws", [1], mybir.dt.float32, isOutput=False)
    y = nc.declare_dram_parameter("y", [m_core, O], mybir.dt.float32, isOutput=True)

    with ExitStack() as ctx:
        tc = ctx.enter_context(tile.TileContext(nc))
        consts = ctx.enter_context(tc.tile_pool(name="consts", bufs=1))
        xpool = ctx.enter_context(tc.tile_pool(name="xin", bufs=4))
        qpool = ctx.enter_context(tc.tile_pool(name="quant", bufs=3))
        tppool = ctx.enter_context(tc.tile_pool(name="xqt", bufs=4))
        spool = ctx.enter_context(tc.tile_pool(name="stats", bufs=6))
        opool = ctx.enter_context(tc.tile_pool(name="yout", bufs=3))
        psy = ctx.enter_context(tc.tile_pool(name="psy", bufs=2, space="PSUM"))

        bias_sb = consts.tile([P, O], mybir.dt.float32)
        nc.sync.dma_start(bias_sb[:], bias[None, :].to_broadcast((P, O)))
        ws_sb = consts.tile([P, 1], mybir.dt.float32)
        nc.sync.dma_start(ws_sb[:], ws[None, :].to_broadcast((P, 1)))
        w_sb = consts.tile([P, NK, O], mybir.dt.bfloat16)
        nc.sync.dma_start(w_sb[:], wT.rearrange("(k p) o -> p k o", p=P))

        x3 = x.rearrange("(t p) d -> t p d", p=P)
        y3 = y.rearrange("(t p) o -> t p o", p=P)

        def emit_front(t):
            """DMA x in, quantize, xbar-transpose -> (xqT tile, rden)."""
            xt = xpool.tile([P, D], mybir.dt.float32, tag="xin")
            nc.sync.dma_start(xt[:], x3[t])

            amax = spool.tile([P, 1], mybir.dt.float32, tag="amax")
            nc.vector.reduce_max(
                amax[:], xt[:], axis=mybir.AxisListType.X,
                apply_absolute_value=True,
            )
            nc.vector.tensor_scalar_max(amax[:], amax[:], 1e-5)
            ramax = spool.tile([P, 1], mybir.dt.float32, tag="ramax")
            nc.vector.reciprocal(ramax[:], amax[:])
            scl = spool.tile([P, 1], mybir.dt.float32, tag="scl")
            nc.vector.tensor_scalar_mul(scl[:], ramax[:], QP)
            den = spool.tile([P, 1], mybir.dt.float32, tag="den")
            nc.vector.tensor_tensor(
                den[:], ws_sb[:], scl[:], mybir.AluOpType.mult
            )
            rden = spool.tile([P, 1], mybir.dt.float32, tag="rden")
            nc.vector.reciprocal(rden[:], den[:])

            # xq = round_half_even(x * scale): DVE mult+MAGIC, ACT -MAGIC->bf16
            t1 = qpool.tile([P, D], mybir.dt.float32, tag="t1")
            nc.vector.tensor_scalar(
                t1[:], xt[:], scl[:], MAGIC,
                op0=mybir.AluOpType.mult, op1=mybir.AluOpType.add,
            )
            xq = qpool.tile([P, D], mybir.dt.bfloat16, tag="xq")
            nc.scalar.activation(
                xq[:], t1[:], mybir.ActivationFunctionType.Copy,
                bias=-MAGIC, scale=1.0,
            )

            # one batched xbar DMA: st[p, k, m] = xq[m, 128k + p]
            st = tppool.tile([P, NK, P], mybir.dt.bfloat16, tag="xqT")
            nc.sync.dma_start_transpose(st[:], xq[:])
            return st, rden

        def emit_tail(t, st, rden):
            ys = [
                psy.tile([P, O_CHUNK], mybir.dt.float32,
                         tag=f"psy{j}", name=f"psy{j}")
                for j in range(N_OCH)
            ]
            for k in range(NK):
                for j in range(N_OCH):
                    nc.tensor.matmul(
                        ys[j][:], st[:, k, :],
                        w_sb[:, k, bass.ts(j, O_CHUNK)],
                        start=(k == 0), stop=(k == NK - 1),
                    )
            yt = opool.tile([P, O], mybir.dt.float32, tag="yt")
            for j in range(N_OCH):
                # dequant fused into PSUM->SBUF copy on ACT
                nc.scalar.activation(
                    yt[:, bass.ts(j, O_CHUNK)], ys[j][:],
                    mybir.ActivationFunctionType.Copy,
                    bias=0.0, scale=rden[:],
                )
            # bias add on Pool (SBUF->SBUF keeps ACT/DVE free)
            nc.gpsimd.tensor_tensor(
                yt[:], yt[:], bias_sb[:], mybir.AluOpType.add
            )
            nc.sync.dma_start(y3[t], yt[:])

        def body(_iv=None):
            pending = []
            for t in range(m_tiles):
                pending.append((t, *emit_front(t)))
                if len(pending) > AHEAD:
                    emit_tail(*pending.pop(0))
            for args in pending:
                emit_tail(*args)

        if repeats == 1:
            body()
        elif repeats > 1:
            with tc.For_i(0, repeats, 1):
                body()
    nc.finalize()
    return nc


def unpack_weights_host(weight_packed):
    wp = np.asarray(weight_packed)
    parts = [((wp >> (2 * i)) & 3) for i in range(4)]
    w = np.concatenate(parts, axis=0).astype(np.float32) - 1.0   # [out, in]
    return np.ascontiguousarray(w.T).astype(ml_dtypes.bfloat16)  # [in, out]


_NC_CACHE = {}


def _get_nc():
    if "nc" not in _NC_CACHE:
        _NC_CACHE["nc"] = build_nc()
    return _NC_CACHE["nc"]


def kernel(x, weight_packed, weight_scale, bias):
    xf = np.ascontiguousarray(np.asarray(x, dtype=np.float32).reshape(M_TOTAL, D))
    wT = unpack_weights_host(weight_packed)
    bias_np = np.ascontiguousarray(np.asarray(bias, dtype=np.float32))
    ws_np = np.ascontiguousarray(np.asarray(weight_scale, dtype=np.float32))

    in_maps = [
        {
            "x": xf[i * M_CORE:(i + 1) * M_CORE],
            "wT": wT,
            "bias": bias_np,
            "ws": ws_np,
        }
        for i in range(N_CORES)
    ]
    res = run_bass_kernel_spmd(_get_nc(), in_maps, list(range(N_CORES))).results
    y = np.concatenate([res[i]["y"] for i in range(N_CORES)], axis=0)
    return np.ascontiguousarray(y.reshape(B, S, O))


# revision 4
# speedup vs baseline: 1.1380x; 1.1380x over previous
"""BitLinear (2-bit packed weights) on 8 Trainium2 NeuronCores.

v1 schedule (PE transposes inline, psy bufs=1) plus
- modulo ring pipeline across For_i iterations (no drain/fill bubble),
- y-out DMA on the ACT hwdge queue (x-in prefetch alone on SP),
- bias-add on the Pool engine.

Math is bit-faithful to the jax reference."""

from contextlib import ExitStack

import numpy as np
import ml_dtypes

import concourse.bass as bass
import concourse.mybir as mybir
import concourse.tile as tile
from concourse import bacc
from concourse.bass_utils import run_bass_kernel_spmd
from concourse.masks import make_identity

P = 128
D = 2048
O = 2048
N_CORES = 8
B, S = 4, 4096
M_TOTAL = B * S
M_CORE = M_TOTAL // N_CORES
NK = D // P
O_CHUNK = 512
N_OCH = O // O_CHUNK
MAGIC = 12582912.0
QP = 127.0
AHEAD = 1


def build_nc(m_core=M_CORE, repeats=1, variant="fused"):
    m_tiles = m_core // P
    nc = bacc.Bacc(None)
    x = nc.declare_dram_parameter("x", [m_core, D], mybir.dt.float32, isOutput=False)
    wT = nc.declare_dram_parameter("wT", [D, O], mybir.dt.bfloat16, isOutput=False)
    bias = nc.declare_dram_parameter("bias", [O], mybir.dt.float32, isOutput=False)
    ws = nc.declare_dram_parameter("ws", [1], mybir.dt.float32, isOutput=False)
    y = nc.declare_dram_parameter("y", [m_core, O], mybir.dt.float32, isOutput=True)

    with ExitStack() as ctx:
        tc = ctx.enter_context(tile.TileContext(nc))
        consts = ctx.enter_context(tc.tile_pool(name="consts", bufs=1))
        xpool = ctx.enter_context(tc.tile_pool(name="xin", bufs=3))
        qpool = ctx.enter_context(tc.tile_pool(name="quant", bufs=2))
        tppool = ctx.enter_context(tc.tile_pool(name="xqt", bufs=12))
        spool = ctx.enter_context(tc.tile_pool(name="stats", bufs=6))
        opool = ctx.enter_context(tc.tile_pool(name="yout", bufs=3))
        psy = ctx.enter_context(tc.tile_pool(name="psy", bufs=1, space="PSUM"))
        pst = ctx.enter_context(tc.tile_pool(name="pst", bufs=3, space="PSUM"))

        ident = consts.tile([P, P], mybir.dt.bfloat16)
        make_identity(nc, ident[:])
        bias_sb = consts.tile([P, O], mybir.dt.float32)
        nc.sync.dma_start(bias_sb[:], bias[None, :].to_broadcast((P, O)))
        ws_sb = consts.tile([P, 1], mybir.dt.float32)
        nc.sync.dma_start(ws_sb[:], ws[None, :].to_broadcast((P, 1)))
        w_sb = consts.tile([P, NK, O], mybir.dt.bfloat16)
        nc.sync.dma_start(w_sb[:], wT.rearrange("(k p) o -> p k o", p=P))

        x3 = x.rearrange("(t p) d -> t p d", p=P)
        y3 = y.rearrange("(t p) o -> t p o", p=P)

        def emit_front(t):
            xt = xpool.tile([P, D], mybir.dt.float32, tag="xin")
            nc.sync.dma_start(xt[:], x3[t])

            amax = spool.tile([P, 1], mybir.dt.float32, tag="amax")
            nc.vector.reduce_max(
                amax[:], xt[:], axis=mybir.AxisListType.X,
                apply_absolute_value=True,
            )
            nc.vector.tensor_scalar_max(amax[:], amax[:], 1e-5)
            ramax = spool.tile([P, 1], mybir.dt.float32, tag="ramax")
            nc.vector.reciprocal(ramax[:], amax[:])
            scl = spool.tile([P, 1], mybir.dt.float32, tag="scl")
            nc.vector.tensor_scalar_mul(scl[:], ramax[:], QP)
            den = spool.tile([P, 1], mybir.dt.float32, tag="den")
            nc.vector.tensor_tensor(
                den[:], ws_sb[:], scl[:], mybir.AluOpType.mult
            )
            rden = spool.tile([P, 1], mybir.dt.float32, tag="rden")
            nc.vector.reciprocal(rden[:], den[:])

            t1 = qpool.tile([P, D], mybir.dt.float32, tag="t1")
            nc.vector.tensor_scalar(
                t1[:], xt[:], scl[:], MAGIC,
                op0=mybir.AluOpType.mult, op1=mybir.AluOpType.add,
            )
            xq = qpool.tile([P, D], mybir.dt.bfloat16, tag="xq")
            nc.scalar.activation(
                xq[:], t1[:], mybir.ActivationFunctionType.Copy,
                bias=-MAGIC, scale=1.0,
            )

            # PE transposes via identity matmul (as v1), PSUM f32 + ACT copy
            xqT = []
            for g in range(NK // 4):
                pt = pst.tile([P, 4 * P], mybir.dt.float32,
                              tag="pst", name=f"pst{g}")
                for kk in range(4):
                    nc.tensor.matmul(
                        pt[:, bass.ts(kk, P)],
                        xq[:, bass.ts(g * 4 + kk, P)], ident[:],
                        start=True, stop=True,
                    )
                st = tppool.tile([P, 4, P], mybir.dt.bfloat16,
                                 tag="xqT", name=f"xqT{g}")
                nc.scalar.copy(
                    st[:], pt[:].rearrange("p (a b) -> p a b", b=P))
                xqT.append(st)
            return xqT, rden

        def emit_tail(t, xqT, rden):
            ys = [
                psy.tile([P, O_CHUNK], mybir.dt.float32,
                         tag=f"psy{j}", name=f"psy{j}")
                for j in range(N_OCH)
            ]
            for k in range(NK):
                for j in range(N_OCH):
                    nc.tensor.matmul(
                        ys[j][:], xqT[k // 4][:, k % 4, :],
                        w_sb[:, k, bass.ts(j, O_CHUNK)],
                        start=(k == 0), stop=(k == NK - 1),
                    )
            yt = opool.tile([P, O], mybir.dt.float32, tag="yt")
            if variant == "fused":
                # one DVE pass per chunk: yt = (psum * rden) + bias
                # (identical roundings to mult-then-add; keeps dequant off
                # the ACT FIFO so MMs never wait behind transpose copies)
                for j in range(N_OCH):
                    nc.vector.scalar_tensor_tensor(
                        yt[:, bass.ts(j, O_CHUNK)], ys[j][:], rden[:],
                        bias_sb[:, bass.ts(j, O_CHUNK)],
                        op0=mybir.AluOpType.mult, op1=mybir.AluOpType.add,
                    )
            else:
                for j in range(N_OCH):
                    nc.scalar.activation(
                        yt[:, bass.ts(j, O_CHUNK)], ys[j][:],
                        mybir.ActivationFunctionType.Copy,
                        bias=0.0, scale=rden[:],
                    )
                nc.gpsimd.tensor_tensor(
                    yt[:], yt[:], bias_sb[:], mybir.AluOpType.add
                )
            nc.scalar.dma_start(y3[t], yt[:])

        fronts = {}
        for t in range(min(AHEAD, m_tiles)):
            fronts[t] = emit_front(t)

        def body(_iv=None):
            for t in range(m_tiles):
                tn = (t + AHEAD) % m_tiles
                fronts[tn] = emit_front(tn)
                emit_tail(t, *fronts.pop(t))

        if repeats == 1:
            body()
        elif repeats > 1:
            with tc.For_i(0, repeats, 1):
                body()
    nc.finalize()
    return nc


def unpack_weights_host(weight_packed):
    wp = np.asarray(weight_packed)
    parts = [((wp >> (2 * i)) & 3) for i in range(4)]
    w = np.concatenate(parts, axis=0).astype(np.float32) - 1.0
    return np.ascontiguousarray(w.T).astype(ml_dtypes.bfloat16)


_NC_CACHE = {}


def _get_nc():
    if "nc" not in _NC_CACHE:
        _NC_CACHE["nc"] = build_nc()
    return _NC_CACHE["nc"]


def kernel(x, weight_packed, weight_scale, bias):
    xf = np.ascontiguousarray(np.asarray(x, dtype=np.float32).reshape(M_TOTAL, D))
    wT = unpack_weights_host(weight_packed)
    bias_np = np.ascontiguousarray(np.asarray(bias, dtype=np.float32))
    ws_np = np.ascontiguousarray(np.asarray(weight_scale, dtype=np.float32))

    in_maps = [
        {
            "x": xf[i * M_CORE:(i + 1) * M_CORE],
            "wT": wT,
            "bias": bias_np,
            "ws": ws_np,
        }
        for i in range(N_CORES)
    ]
    res = run_bass_kernel_spmd(_get_nc(), in_maps, list(range(N_CORES))).results
    y = np.concatenate([res[i]["y"] for i in range(N_CORES)], axis=0)
    return np.ascontiguousarray(y.reshape(B, S, O))
